# revision 53
# baseline (speedup 1.0000x reference)
"""Multi-head attention (B=4, T=2048, D=768, H=12) on 8 NeuronCores.

Sharding: core c handles batch b = c//2 and head-group g = c%2 (heads
6g..6g+5).  Each core computes its 6 heads' attention and a partial
output projection (contraction over its 384 local dims of w_proj); the
host sums the two partials per batch and adds the bias row.

Device formulation (bf16 matmul operands everywhere, fp32 psum):
  qT = Wq'.T @ xT  [384, 2048]   (Wq' pre-scaled by 1/sqrt(hd) on host)
  kT = Wk.T @ xT   [384, 2048]
  v  = x @ Wv      [2048, 384]   per kpos tile (65th column = 1.0)
  S^T[kt] = kT_h.T @ qT_h   [128 kpos, 1024 q]  per head, q-slab halves
  P^T = exp(S^T)   (ACT; scores max ~8 so no max subtraction)
  O[q, 65] += P^T[kt].T @ v'[kt]   <- flipped: output partitions = 128 q
      (col 64 accumulates the softmax denominators via the ones column)
  o = O[:, 0:64] * rcp(O[:, 64])   fused normalize in the psum drain
  oT via PE transpose (identity stationary), y = oT.T @ Wp per q tile.

The flip halves the P@V matmul cost vs the [65, 512]-output orientation
(the timeline cost model charges out-free-size cycles per matmul, so
output partition utilization is what matters).  Schedule: ACT (exp)
paces the attention inner loop at ~1.04us per [128,1024] tile; S, O,
QKV-projection, V, transposes and the output projection are spread
across the 12 (head, q-slab) sweeps to keep PE under that pace.
O-matmuls trail their exp by 4 kt iterations so psum-slot drains (DVE)
never stall the PE queue head.
"""

import numpy as np

EMBED = 768
HEADS = 12
HD = 64
SCALE = HD ** -0.5
B, T = 4, 2048
NCORES = 8
HPC = 6            # heads per core
DL = HPC * HD      # 384 local model dims per core

_prog_cache = {}


def _build_program(repeat=1):
    import concourse.bass as bass
    import concourse.mybir as mybir
    import concourse.tile as tile
    from concourse import bacc

    f32 = mybir.dt.float32
    f32r = mybir.dt.float32r
    bf16 = mybir.dt.bfloat16
    ACT_EXP = mybir.ActivationFunctionType.Exp
    ACT_COPY = mybir.ActivationFunctionType.Copy

    nc = bacc.Bacc()

    xt_d = nc.dram_tensor("xt", [EMBED, T], bf16, kind="ExternalInput")
    wq_d = nc.dram_tensor("wq", [EMBED, DL], bf16, kind="ExternalInput")
    wk_d = nc.dram_tensor("wk", [EMBED, DL], bf16, kind="ExternalInput")
    wv_d = nc.dram_tensor("wv", [EMBED, DL], bf16, kind="ExternalInput")
    bqs_d = nc.dram_tensor("bqs", [DL], f32, kind="ExternalInput")
    bk_d = nc.dram_tensor("bk", [DL], f32, kind="ExternalInput")
    wp_d = nc.dram_tensor("wp", [DL, EMBED], bf16, kind="ExternalInput")
    id_d = nc.dram_tensor("ident", [128, 128], f32r, kind="ExternalInput")
    y_d = nc.dram_tensor("y", [T, EMBED], bf16, kind="ExternalOutput")

    NDT = EMBED // 128   # 6 contraction tiles over embed dim
    NKT = T // 128       # 16 key-position tiles
    NQT = T // 128       # 16 query row tiles
    LAG = 11             # O-matmul lag (in kt iterations) behind exp: each
    #                      slab's kt5..15 O-matmuls spill into the next slab,
    #                      spreading V/QKV pressure out of the first slabs

    # (head, q-slab) sweep order: q-major within each head pair so a
    # pair's q-half completes as early as possible (feeds transposes).
    SLABS = [(0, 0), (1, 0), (0, 1), (1, 1),
             (2, 0), (3, 0), (2, 1), (3, 1),
             (4, 0), (5, 0), (4, 1), (5, 1)]

    with tile.TileContext(nc) as tc:
      for _rep in range(repeat):
        with tc.tile_pool(name="pers", bufs=1) as pers, \
             tc.tile_pool(name="qk", bufs=2) as qk_pool, \
             tc.tile_pool(name="pt", bufs=2) as pt_pool, \
             tc.tile_pool(name="rcp", bufs=4) as rcp_pool, \
             tc.tile_pool(name="yr", bufs=3) as yr_pool, \
             tc.tile_pool(name="pss", bufs=2, space="PSUM") as pss_pool, \
             tc.tile_pool(name="po", bufs=2, space="PSUM") as po_pool, \
             tc.tile_pool(name="aux", bufs=2, space="PSUM") as aux_pool:

            xt_dts = [pers.tile([128, T], bf16, name=f"xt{dt}_sb")
                      for dt in range(NDT)]
            wq_sb = pers.tile([128, NDT, DL], bf16, name="wq_sb")
            wk_sb = pers.tile([128, NDT, DL], bf16, name="wk_sb")
            wv_sb = pers.tile([128, NDT, DL], bf16, name="wv_sb")
            wp_sb = pers.tile([128, 3, EMBED], bf16, name="wp_sb")
            v_sb = pers.tile([128, NKT, HPC, HD + 1], bf16, name="v_sb")
            bqs_sb = pers.tile([128, 3], f32, name="bqs_sb")
            bk_sb = pers.tile([128, 3], f32, name="bk_sb")
            o_sb = pers.tile([128, NQT, 3, 128], bf16, name="o_sb")
            # pair-2 qh1 stages in f32r so the tail can transpose on the PE
            # (no serial HWDGE descriptor-gens on the critical tail path)
            o2q1_sb = pers.tile([128, 8, 128], f32r, name="o2q1_sb")
            id_sb = pers.tile([128, 128], f32r, name="id_sb")
            oT_sb = pers.tile([128, 3, T], bf16, name="oT_sb")
            warm_sb = pers.tile([128, 256], bf16, name="warm_sb")

            # ones column of v' (softmax denominator accumulator) — only
            # the 65th columns; emitted first so it doesn't sit behind the
            # DMA descriptor generation on the Pool queue
            nc.gpsimd.memset(v_sb[:, :, :, HD:HD + 1], 1.0)
            nc.vector.memset(warm_sb, 0.0)

            # input DMAs: first-slab critical path is wk/wq + all of xt
            # (full embed contraction).  The DMA copies serialize on one
            # resource, so everything not needed before the first S goes
            # after xt.  Per-dt xt tiles give each transfer its own
            # completion sem (DMA write deps are tile x queue granular).
            nc.gpsimd.dma_start(out=wk_sb, in_=wk_d.ap().rearrange("(n p) m -> p n m", p=128))
            nc.gpsimd.dma_start(out=wq_sb, in_=wq_d.ap().rearrange("(n p) m -> p n m", p=128))
            for dt in range(3):
                nc.sync.dma_start(out=xt_dts[dt], in_=xt_d.ap()[bass.ts(dt, 128), :])
            for dt in range(3, NDT):
                nc.gpsimd.dma_start(out=xt_dts[dt], in_=xt_d.ap()[bass.ts(dt, 128), :])
            nc.gpsimd.dma_start(out=wv_sb, in_=wv_d.ap().rearrange("(n p) m -> p n m", p=128))
            nc.gpsimd.dma_start(out=wp_sb, in_=wp_d.ap().rearrange("(n p) m -> p n m", p=128))
            nc.gpsimd.dma_start(out=id_sb, in_=id_d.ap())
            nc.sync.dma_start(out=bqs_sb, in_=bqs_d.ap().rearrange("(n p) -> p n", p=128))
            nc.sync.dma_start(out=bk_sb, in_=bk_d.ap().rearrange("(n p) -> p n", p=128))

            def warm(n):
                # warm-up matmuls ride the po slots (idle until the first
                # slab's O accumulation; pss holds ps_q01 through startup)
                for _w in range(n):
                    psw = po_pool.tile([128, 256], f32, name="psw", tag="po")
                    nc.tensor.matmul(psw, warm_sb[0:2, 0:128], warm_sb[0:2, :],
                                     start=True, stop=True)

            warm(6)

            qk_tiles = {}
            yr_tiles = {}

            def mk_pair(hp):
                qk_tiles[hp] = (
                    qk_pool.tile([128, T], bf16, name="qTp", tag="qT"),
                    qk_pool.tile([128, T], bf16, name="kTp", tag="kT"),
                )

            def qkv_group(hp, ch, which):
                csl = bass.ts(ch, 512)
                qTp, kTp = qk_tiles[hp]
                dst, wsb, bias = (
                    (qTp, wq_sb, bqs_sb) if which == "q" else (kTp, wk_sb, bk_sb)
                )
                ps = aux_pool.tile([128, 512], f32, name="psqk", tag="aux")
                for dt in range(NDT):
                    nc.tensor.matmul(
                        ps, wsb[:, dt, bass.ts(hp, 128)], xt_dts[dt][:, csl],
                        start=(dt == 0), stop=(dt == NDT - 1),
                    )
                nc.vector.tensor_scalar_add(
                    out=dst[:, csl], in0=ps, scalar1=bias[:, hp:hp + 1],
                )

            def v_emit(kt):
                ps = aux_pool.tile([128, DL], f32, name="psv", tag="aux")
                for dt in range(NDT):
                    nc.tensor.matmul(
                        ps, xt_dts[dt][:, bass.ts(kt, 128)], wv_sb[:, dt, :],
                        start=(dt == 0), stop=(dt == NDT - 1),
                    )
                # GPSIMD cannot touch PSUM; DVE is nearly idle during the
                # V-emission slab (ACT copies here would stall its in-order
                # queue ahead of the exps)
                nc.vector.tensor_copy(
                    out=v_sb[:, kt, :, 0:HD],
                    in_=ps.rearrange("p (h d) -> p h d", h=HPC),
                )

            def transpose_qt(pair, qt, tail=False):
                # 2-byte dtypes transpose on the DMA xbar (PE transpose into
                # psum is 4-byte-cell granular and corrupts bf16).  Tail
                # transposes issue from the otherwise-idle ACT queue so their
                # descriptor generation doesn't serialize behind the y DMAs.
                eng = nc.scalar if tail else nc.sync
                eng.dma_start_transpose(
                    out=oT_sb[:, pair, bass.ts(qt, 128)],
                    in_=o_sb[:, qt, pair, :],
                )

            def proj_group(qt, nh, tail=False):
                ps = aux_pool.tile([128, 384], f32, name="psy", tag="aux")
                for dtp in range(3):
                    nc.tensor.matmul(
                        ps, oT_sb[:, dtp, bass.ts(qt, 128)],
                        wp_sb[:, dtp, bass.ts(nh, 384)],
                        start=(dtp == 0), stop=(dtp == 2),
                    )
                if nh == 0:
                    yr = yr_pool.tile([128, EMBED], bf16, name="yr", tag="yr")
                    yr_tiles[qt] = yr
                    nc.vector.tensor_copy(out=yr[:, 0:384], in_=ps)
                else:
                    yr = yr_tiles.pop(qt)
                    if tail:  # ACT is idle once attention has drained
                        nc.scalar.activation(out=yr[:, 384:768], in_=ps, func=ACT_COPY)
                    else:
                        nc.vector.tensor_copy(out=yr[:, 384:768], in_=ps)
                    nc.sync.dma_start(out=y_d.ap()[bass.ts(qt, 128), :], in_=yr)

            def drain_po(h, qh, po, qt_base):
                # fused normalize: o = O[:, 0:64] / O[:, 64] at psum drain
                pair, off = h // 2, (h % 2) * HD
                rcp = rcp_pool.tile([128, 4], f32, name="rcp", tag="rcp")
                nc.vector.reciprocal(out=rcp, in_=po[:, :, HD])
                for j in range(4):
                    qt = qh * 8 + qt_base + j
                    if pair == 2 and qh == 1:
                        dst = o2q1_sb[:, qt - 8, off:off + HD]
                    else:
                        dst = o_sb[:, qt, pair, off:off + HD]
                    with nc.allow_low_precision(reason="f32r staging"):
                        nc.vector.tensor_scalar_mul(
                            out=dst, in0=po[:, j, 0:HD], scalar1=rcp[:, j:j + 1],
                        )

            spill = []   # closures: previous slab's trailing O-matmuls + drains

            def attend(h, qh, fillers, late_fillers=(), last=False):
                nonlocal spill
                hp, off = h // 2, (h % 2) * HD
                qTp, kTp = qk_tiles[hp]
                pts = pt_pool.tile([128, NKT, 1024], bf16, name="pts", tag="pt")
                po_t = [None, None]
                myspill = []

                def own_o(kt):
                    for qt in range(8):
                        po = po_t[qt // 4]
                        # start=True zeroes the whole 2KB psum bank, so only
                        # the first column of each po bank may assert it
                        nc.tensor.matmul(
                            po[:, qt % 4, :],
                            pts[:, kt, bass.ts(qt, 128)],
                            v_sb[:, kt, h, :],
                            start=(kt == 0 and qt % 4 == 0),
                            stop=(kt == NKT - 1),
                            skip_group_check=True,
                        )

                # early fillers (V / QKV groups — read no drain-produced
                # tiles) spread over j=0..LAG-1; late fillers (transposes,
                # proj — emission-ordered after the j=9,10 drains) over the
                # remaining iterations
                fi = [0, 0]
                flists = (fillers, late_fillers)
                spans = ((0, LAG), (LAG, NKT))

                def emit_fillers(j, which):
                    lo, hi = spans[which]
                    if j < lo:
                        return
                    fl = flists[which]
                    upto = min(
                        (len(fl) * (j - lo + 1) + (hi - lo) - 1) // (hi - lo),
                        len(fl),
                    )
                    while fi[which] < upto:
                        fl[fi[which]]()
                        fi[which] += 1

                for j in range(NKT):
                    pss = pss_pool.tile([128, 1024], f32, name="pss", tag="pss")
                    for c2 in range(2):
                        nc.tensor.matmul(
                            pss[:, bass.ts(c2, 512)],
                            kTp[off:off + HD, bass.ts(j, 128)],
                            qTp[off:off + HD, bass.ds(qh * 1024 + c2 * 512, 512)],
                            start=True, stop=True,
                        )
                    nc.scalar.activation(out=pts[:, j, :], in_=pss, func=ACT_EXP)
                    if j < len(spill):
                        spill[j]()
                    if j == LAG:
                        po_t[0] = po_pool.tile([128, 4, HD + 1], f32, name="po0", tag="po")
                        po_t[1] = po_pool.tile([128, 4, HD + 1], f32, name="po1", tag="po")
                    if j >= LAG:
                        own_o(j - LAG)
                        if last:
                            # no successor S-stream to protect: double-rate
                            # the trailing O-matmuls to shrink the tail
                            own_o(j - LAG + 5)
                    emit_fillers(j, 0)
                    emit_fillers(j, 1)

                if last:
                    # in-loop covered kt0..9 (double rate); tail gets 10..15
                    for kt in range(10, NKT):
                        myspill.append(lambda kt=kt: own_o(kt))
                else:
                    # trailing O-matmuls kt5..15 spill into the next slab at
                    # j=0..8 (two-per-j for the first two), drains at j=9,10
                    # — safely before the po slots are reallocated at j=11
                    myspill.append(lambda: (own_o(NKT - LAG), own_o(NKT - LAG + 1)))
                    myspill.append(lambda: (own_o(NKT - LAG + 2), own_o(NKT - LAG + 3)))
                    for kt in range(NKT - LAG + 4, NKT):
                        myspill.append(lambda kt=kt: own_o(kt))
                myspill.append(lambda: drain_po(h, qh, po_t[0], 0))
                myspill.append(lambda: drain_po(h, qh, po_t[1], 4))
                spill = myspill

            # ---- static filler schedule -------------------------------
            mk_pair(0)
            FE = {s: [] for s in range(1, 13)}   # early: V / QKV groups
            FL = {s: [] for s in range(1, 13)}   # late: transposes / proj
            FE[1] = (
                [lambda: qkv_group(0, 1, "k")]
                + [lambda kt=kt: v_emit(kt) for kt in range(2)]
                + [lambda: qkv_group(0, 2, "k")]
                + [lambda kt=kt: v_emit(kt) for kt in range(2, 4)]
                + [lambda: qkv_group(0, 3, "k")]
                + [lambda kt=kt: v_emit(kt) for kt in range(4, 9)]
            )
            FE[2] = (
                [lambda kt=kt: v_emit(kt) for kt in range(9, 16)]
                + [lambda: qkv_group(0, 2, "q"),
                   lambda: qkv_group(0, 3, "q")]
            )
            FE[3] = [
                lambda: mk_pair(1),
                lambda: qkv_group(1, 0, "k"),
                lambda: qkv_group(1, 0, "q"),
                lambda: qkv_group(1, 1, "k"),
                lambda: qkv_group(1, 1, "q"),
            ]
            FL[3] = [lambda qt=qt: transpose_qt(0, qt) for qt in range(8)]
            FE[4] = [
                lambda: qkv_group(1, 2, "k"),
                lambda: qkv_group(1, 2, "q"),
                lambda: qkv_group(1, 3, "k"),
                lambda: qkv_group(1, 3, "q"),
            ]
            FE[5] = [
                lambda: mk_pair(2),
                lambda: qkv_group(2, 0, "k"),
                lambda: qkv_group(2, 0, "q"),
            ]
            FL[5] = [lambda qt=qt: transpose_qt(0, qt) for qt in range(8, 16)]
            FE[6] = [
                lambda: qkv_group(2, 1, "k"),
                lambda: qkv_group(2, 1, "q"),
            ]
            FL[7] = [lambda qt=qt: transpose_qt(1, qt) for qt in range(8)]
            FE[7] = [
                lambda: qkv_group(2, 2, "k"),
                lambda: qkv_group(2, 2, "q"),
            ]
            FE[8] = [
                lambda: qkv_group(2, 3, "k"),
                lambda: qkv_group(2, 3, "q"),
            ]
            FL[9] = [lambda qt=qt: transpose_qt(1, qt) for qt in range(8, 16)]
            FL[11] = (
                [lambda qt=qt: transpose_qt(2, qt) for qt in range(8)]
                + [lambda qt=qt, nh=nh: proj_group(qt, nh)
                   for qt in range(4) for nh in range(2)]
            )
            FL[12] = [lambda qt=qt, nh=nh: proj_group(qt, nh)
                      for qt in range(4, 8) for nh in range(2)]

            # ---- startup: pair-0 chunks needed by the first slab, with
            # the three accumulations interleaved by dt so each matmul runs
            # as its xt tile lands (q-ch1 borrows a po slot; aux has 2)
            qTp0, kTp0 = qk_tiles[0]
            ps_k0 = aux_pool.tile([128, 512], f32, name="ps_k0", tag="aux")
            ps_q01 = pss_pool.tile([128, 1024], f32, name="ps_q01", tag="pss")
            for dt in range(NDT):
                nc.tensor.matmul(ps_k0, wk_sb[:, dt, 0:128], xt_dts[dt][:, 0:512],
                                 start=(dt == 0), stop=(dt == NDT - 1))
                nc.tensor.matmul(ps_q01[:, 0:512], wq_sb[:, dt, 0:128],
                                 xt_dts[dt][:, 0:512],
                                 start=(dt == 0), stop=(dt == NDT - 1))
                nc.tensor.matmul(ps_q01[:, 512:1024], wq_sb[:, dt, 0:128],
                                 xt_dts[dt][:, 512:1024],
                                 start=(dt == 0), stop=(dt == NDT - 1))
                if dt < NDT - 1:
                    warm(3)
            # drain the first kt tile's kT columns first so S(kt0) can
            # issue as early as possible, then the q chunks, then the rest
            nc.vector.tensor_scalar_add(out=kTp0[:, 0:128], in0=ps_k0[:, 0:128],
                                        scalar1=bk_sb[:, 0:1])
            nc.vector.tensor_scalar_add(out=qTp0[:, 0:1024], in0=ps_q01,
                                        scalar1=bqs_sb[:, 0:1])
            nc.vector.tensor_scalar_add(out=kTp0[:, 128:512], in0=ps_k0[:, 128:512],
                                        scalar1=bk_sb[:, 0:1])

            for s, (h, qh) in enumerate(SLABS, start=1):
                attend(h, qh, FE[s], FL[s], last=(s == 12))

            # ---- tail -------------------------------------------------
            for fn in spill:
                fn()
            # tail: PE transposes (f32r) run two qt ahead of the projection
            # so the PE→DVE→PE chain pipelines (no HWDGE gen on the
            # critical path); full-width proj psum rides the freed pss
            # slots and the two yr halves drain on DVE and ACT in parallel
            def tail_transpose(qt):
                pst = aux_pool.tile([128, 128], f32r, name="pst", tag="aux")
                nc.tensor.matmul(pst, o2q1_sb[:, qt - 8, :], id_sb,
                                 is_transpose=True)
                nc.vector.tensor_copy(out=oT_sb[:, 2, bass.ts(qt, 128)], in_=pst)

            tail_transpose(8)
            tail_transpose(9)
            for qt in range(8, 16):
                if qt + 2 < 16:
                    tail_transpose(qt + 2)
                # [128, 2, 512]: each nh half owns a full psum bank (matmul
                # accumulation groups must not cross bank boundaries)
                ps = pss_pool.tile([128, 2, 512], f32, name="psyt", tag="pss")
                for nh in range(2):
                    for dtp in range(3):
                        nc.tensor.matmul(
                            ps[:, nh, 0:384], oT_sb[:, dtp, bass.ts(qt, 128)],
                            wp_sb[:, dtp, bass.ts(nh, 384)],
                            start=(dtp == 0), stop=(dtp == 2),
                        )
                yr = yr_pool.tile([128, EMBED], bf16, name="yr", tag="yr")
                nc.vector.tensor_copy(out=yr[:, 0:384], in_=ps[:, 0, 0:384])
                nc.scalar.activation(out=yr[:, 384:768], in_=ps[:, 1, 0:384],
                                     func=ACT_COPY)
                nc.sync.dma_start(out=y_d.ap()[bass.ts(qt, 128), :], in_=yr)

    nc.finalize()
    return nc


def _shard_inputs(x, w_qkv, b_qkv, w_proj):
    import ml_dtypes

    bf16 = ml_dtypes.bfloat16
    in_maps = []
    for c in range(NCORES):
        b, g = c // 2, c % 2
        sl = slice(DL * g, DL * g + DL)
        in_maps.append({
            "xt": np.ascontiguousarray(x[b].T).astype(bf16),
            "wq": np.ascontiguousarray(w_qkv[:, sl] * SCALE).astype(bf16),
            "wk": np.ascontiguousarray(w_qkv[:, EMBED:][:, sl]).astype(bf16),
            "wv": np.ascontiguousarray(w_qkv[:, 2 * EMBED:][:, sl]).astype(bf16),
            "bqs": np.ascontiguousarray(b_qkv[sl] * SCALE),
            "bk": np.ascontiguousarray(b_qkv[EMBED:][sl]),
            "wp": np.ascontiguousarray(w_proj[sl, :]).astype(bf16),
            "ident": np.eye(128, dtype=np.float32),
        })
    return in_maps


def kernel(x, w_qkv, b_qkv, w_proj, b_proj, _profile=False, _repeat=1):
    from concourse.bass_utils import run_bass_kernel_spmd

    x = np.asarray(x, dtype=np.float32)
    w_qkv = np.asarray(w_qkv, dtype=np.float32)
    b_qkv = np.asarray(b_qkv, dtype=np.float32)
    w_proj = np.asarray(w_proj, dtype=np.float32)
    b_proj = np.asarray(b_proj, dtype=np.float32)

    if _repeat not in _prog_cache:
        _prog_cache[_repeat] = _build_program(_repeat)
    nc = _prog_cache[_repeat]

    in_maps = _shard_inputs(x, w_qkv, b_qkv, w_proj)
    res = run_bass_kernel_spmd(
        nc, in_maps, list(range(NCORES)), trace=_profile,
    )

    # host-side gather: sum the two head-group partials per batch and add
    # the bias row (v-bias folded through w_proj, plus b_proj itself)
    bias_row = b_qkv[2 * EMBED:] @ w_proj + b_proj
    y = np.empty((B, T, EMBED), dtype=np.float32)
    for b in range(B):
        y[b] = (res.results[2 * b]["y"].astype(np.float32)
                + res.results[2 * b + 1]["y"].astype(np.float32) + bias_row)
    if _profile:
        return y, res
    return y


# revision 56
# speedup vs baseline: 1.0044x; 1.0044x over previous
"""Multi-head attention (B=4, T=2048, D=768, H=12) on 8 NeuronCores.

Sharding: core c handles batch b = c//2 and head-group g = c%2 (heads
6g..6g+5).  Each core computes its 6 heads' attention and a partial
output projection (contraction over its 384 local dims of w_proj); the
host sums the two partials per batch and adds the bias row.

Device formulation (bf16 matmul operands everywhere, fp32 psum):
  qT = Wq'.T @ xT  [384, 2048]   (Wq' pre-scaled by 1/sqrt(hd) on host)
  kT = Wk.T @ xT   [384, 2048]
  v  = x @ Wv      [2048, 384]   per kpos tile (65th column = 1.0)
  S^T[kt] = kT_h.T @ qT_h   [128 kpos, 1024 q]  per head, q-slab halves
  P^T = exp(S^T)   (ACT; scores max ~8 so no max subtraction)
  O[q, 65] += P^T[kt].T @ v'[kt]   <- flipped: output partitions = 128 q
      (col 64 accumulates the softmax denominators via the ones column)
  o = O[:, 0:64] * rcp(O[:, 64])   fused normalize in the psum drain
  oT via PE transpose (identity stationary), y = oT.T @ Wp per q tile.

The flip halves the P@V matmul cost vs the [65, 512]-output orientation
(the timeline cost model charges out-free-size cycles per matmul, so
output partition utilization is what matters).  Schedule: ACT (exp)
paces the attention inner loop at ~1.04us per [128,1024] tile; S, O,
QKV-projection, V, transposes and the output projection are spread
across the 12 (head, q-slab) sweeps to keep PE under that pace.
O-matmuls trail their exp by 4 kt iterations so psum-slot drains (DVE)
never stall the PE queue head.
"""

import numpy as np

EMBED = 768
HEADS = 12
HD = 64
SCALE = HD ** -0.5
B, T = 4, 2048
NCORES = 8
HPC = 6            # heads per core
DL = HPC * HD      # 384 local model dims per core

_prog_cache = {}


def _build_program(repeat=1):
    import concourse.bass as bass
    import concourse.mybir as mybir
    import concourse.tile as tile
    from concourse import bacc

    f32 = mybir.dt.float32
    f32r = mybir.dt.float32r
    bf16 = mybir.dt.bfloat16
    ACT_EXP = mybir.ActivationFunctionType.Exp
    ACT_COPY = mybir.ActivationFunctionType.Copy

    nc = bacc.Bacc()

    xt_d = nc.dram_tensor("xt", [EMBED, T], bf16, kind="ExternalInput")
    wq_d = nc.dram_tensor("wq", [EMBED, DL], bf16, kind="ExternalInput")
    wk_d = nc.dram_tensor("wk", [EMBED, DL], bf16, kind="ExternalInput")
    wv_d = nc.dram_tensor("wv", [EMBED, DL], bf16, kind="ExternalInput")
    bqs_d = nc.dram_tensor("bqs", [DL], f32, kind="ExternalInput")
    bk_d = nc.dram_tensor("bk", [DL], f32, kind="ExternalInput")
    wp_d = nc.dram_tensor("wp", [DL, EMBED], bf16, kind="ExternalInput")
    id_d = nc.dram_tensor("ident", [128, 128], f32r, kind="ExternalInput")
    y_d = nc.dram_tensor("y", [T, EMBED], bf16, kind="ExternalOutput")

    NDT = EMBED // 128   # 6 contraction tiles over embed dim
    NKT = T // 128       # 16 key-position tiles
    NQT = T // 128       # 16 query row tiles
    LAG = 11             # O-matmul lag (in kt iterations) behind exp: each
    #                      slab's kt5..15 O-matmuls spill into the next slab,
    #                      spreading V/QKV pressure out of the first slabs

    # (head, q-slab) sweep order: q-major within each head pair so a
    # pair's q-half completes as early as possible (feeds transposes).
    SLABS = [(0, 0), (1, 0), (0, 1), (1, 1),
             (2, 0), (3, 0), (2, 1), (3, 1),
             (4, 0), (5, 0), (4, 1), (5, 1)]

    with tile.TileContext(nc) as tc:
      for _rep in range(repeat):
        with tc.tile_pool(name="pers", bufs=1) as pers, \
             tc.tile_pool(name="qk", bufs=2) as qk_pool, \
             tc.tile_pool(name="pt", bufs=2) as pt_pool, \
             tc.tile_pool(name="rcp", bufs=4) as rcp_pool, \
             tc.tile_pool(name="yr", bufs=3) as yr_pool, \
             tc.tile_pool(name="pss", bufs=2, space="PSUM") as pss_pool, \
             tc.tile_pool(name="po", bufs=2, space="PSUM") as po_pool, \
             tc.tile_pool(name="aux", bufs=2, space="PSUM") as aux_pool:

            xt_dts = [pers.tile([128, T], bf16, name=f"xt{dt}_sb")
                      for dt in range(NDT)]
            wq_sb = pers.tile([128, NDT, DL], bf16, name="wq_sb")
            wk_sb = pers.tile([128, NDT, DL], bf16, name="wk_sb")
            wv_sb = pers.tile([128, NDT, DL], bf16, name="wv_sb")
            wp_sb = pers.tile([128, 3, EMBED], bf16, name="wp_sb")
            v_sb = pers.tile([128, NKT, HPC, HD + 1], bf16, name="v_sb")
            bqs_sb = pers.tile([128, 3], f32, name="bqs_sb")
            bk_sb = pers.tile([128, 3], f32, name="bk_sb")
            o_sb = pers.tile([128, NQT, 3, 128], bf16, name="o_sb")
            # pair-2 qh1 stages in f32r so the tail can transpose on the PE
            # (no serial HWDGE descriptor-gens on the critical tail path)
            o2q1_sb = pers.tile([128, 8, 128], f32r, name="o2q1_sb")
            id_sb = pers.tile([128, 128], f32r, name="id_sb")
            oT_sb = pers.tile([128, 3, T], bf16, name="oT_sb")
            warm_sb = pers.tile([128, 256], bf16, name="warm_sb")

            # ones column of v' (softmax denominator accumulator) — only
            # the 65th columns; emitted first so it doesn't sit behind the
            # DMA descriptor generation on the Pool queue
            nc.gpsimd.memset(v_sb[:, :, :, HD:HD + 1], 1.0)
            nc.vector.memset(warm_sb, 0.0)

            # input DMAs: first-slab critical path is wk/wq + all of xt
            # (full embed contraction).  The DMA copies serialize on one
            # resource, so everything not needed before the first S goes
            # after xt.  Per-dt xt tiles give each transfer its own
            # completion sem (DMA write deps are tile x queue granular).
            nc.gpsimd.dma_start(out=wk_sb, in_=wk_d.ap().rearrange("(n p) m -> p n m", p=128))
            nc.gpsimd.dma_start(out=wq_sb, in_=wq_d.ap().rearrange("(n p) m -> p n m", p=128))
            for dt in range(3):
                nc.sync.dma_start(out=xt_dts[dt], in_=xt_d.ap()[bass.ts(dt, 128), :])
            for dt in range(3, NDT):
                nc.gpsimd.dma_start(out=xt_dts[dt], in_=xt_d.ap()[bass.ts(dt, 128), :])
            nc.gpsimd.dma_start(out=wv_sb, in_=wv_d.ap().rearrange("(n p) m -> p n m", p=128))
            nc.gpsimd.dma_start(out=wp_sb, in_=wp_d.ap().rearrange("(n p) m -> p n m", p=128))
            nc.gpsimd.dma_start(out=id_sb, in_=id_d.ap())
            nc.sync.dma_start(out=bqs_sb, in_=bqs_d.ap().rearrange("(n p) -> p n", p=128))
            nc.sync.dma_start(out=bk_sb, in_=bk_d.ap().rearrange("(n p) -> p n", p=128))

            def warm(n):
                # warm-up matmuls ride the po slots (idle until the first
                # slab's O accumulation; pss holds ps_q01 through startup)
                for _w in range(n):
                    psw = po_pool.tile([128, 256], f32, name="psw", tag="po")
                    nc.tensor.matmul(psw, warm_sb[0:2, 0:128], warm_sb[0:2, :],
                                     start=True, stop=True)

            warm(6)

            qk_tiles = {}
            yr_tiles = {}

            def mk_pair(hp):
                qk_tiles[hp] = (
                    qk_pool.tile([128, T], bf16, name="qTp", tag="qT"),
                    qk_pool.tile([128, T], bf16, name="kTp", tag="kT"),
                )

            def qkv_group(hp, ch, which):
                csl = bass.ts(ch, 512)
                qTp, kTp = qk_tiles[hp]
                dst, wsb, bias = (
                    (qTp, wq_sb, bqs_sb) if which == "q" else (kTp, wk_sb, bk_sb)
                )
                ps = aux_pool.tile([128, 512], f32, name="psqk", tag="aux")
                for dt in range(NDT):
                    nc.tensor.matmul(
                        ps, wsb[:, dt, bass.ts(hp, 128)], xt_dts[dt][:, csl],
                        start=(dt == 0), stop=(dt == NDT - 1),
                    )
                nc.vector.tensor_scalar_add(
                    out=dst[:, csl], in0=ps, scalar1=bias[:, hp:hp + 1],
                )

            def v_emit(kt):
                ps = aux_pool.tile([128, DL], f32, name="psv", tag="aux")
                for dt in range(NDT):
                    nc.tensor.matmul(
                        ps, xt_dts[dt][:, bass.ts(kt, 128)], wv_sb[:, dt, :],
                        start=(dt == 0), stop=(dt == NDT - 1),
                    )
                # GPSIMD cannot touch PSUM; DVE is nearly idle during the
                # V-emission slab (ACT copies here would stall its in-order
                # queue ahead of the exps)
                nc.vector.tensor_copy(
                    out=v_sb[:, kt, :, 0:HD],
                    in_=ps.rearrange("p (h d) -> p h d", h=HPC),
                )

            def transpose_qt(pair, qt, tail=False):
                # 2-byte dtypes transpose on the DMA xbar (PE transpose into
                # psum is 4-byte-cell granular and corrupts bf16).  Tail
                # transposes issue from the otherwise-idle ACT queue so their
                # descriptor generation doesn't serialize behind the y DMAs.
                eng = nc.scalar if tail else nc.sync
                eng.dma_start_transpose(
                    out=oT_sb[:, pair, bass.ts(qt, 128)],
                    in_=o_sb[:, qt, pair, :],
                )

            def proj_group(qt, nh, tail=False):
                ps = aux_pool.tile([128, 384], f32, name="psy", tag="aux")
                for dtp in range(3):
                    nc.tensor.matmul(
                        ps, oT_sb[:, dtp, bass.ts(qt, 128)],
                        wp_sb[:, dtp, bass.ts(nh, 384)],
                        start=(dtp == 0), stop=(dtp == 2),
                    )
                if nh == 0:
                    yr = yr_pool.tile([128, EMBED], bf16, name="yr", tag="yr")
                    yr_tiles[qt] = yr
                    nc.vector.tensor_copy(out=yr[:, 0:384], in_=ps)
                else:
                    yr = yr_tiles.pop(qt)
                    if tail:  # ACT is idle once attention has drained
                        nc.scalar.activation(out=yr[:, 384:768], in_=ps, func=ACT_COPY)
                    else:
                        nc.vector.tensor_copy(out=yr[:, 384:768], in_=ps)
                    nc.sync.dma_start(out=y_d.ap()[bass.ts(qt, 128), :], in_=yr)

            def drain_po(h, qh, po, qt_base, tail=False):
                # fused normalize: o = O[:, 0:64] / O[:, 64] at psum drain.
                # In the tail (ACT idle) half the copies go through ACT's
                # Copy-with-scale to unserialize the DVE.
                pair, off = h // 2, (h % 2) * HD
                rcp = rcp_pool.tile([128, 4], f32, name="rcp", tag="rcp")
                nc.vector.reciprocal(out=rcp, in_=po[:, :, HD])
                for j in range(4):
                    qt = qh * 8 + qt_base + j
                    if pair == 2 and qh == 1:
                        dst = o2q1_sb[:, qt - 8, off:off + HD]
                    else:
                        dst = o_sb[:, qt, pair, off:off + HD]
                    if tail and j % 2 == 1:
                        nc.scalar.activation(
                            out=dst, in_=po[:, j, 0:HD], func=ACT_COPY,
                            scale=rcp[:, j:j + 1],
                        )
                    else:
                        with nc.allow_low_precision(reason="f32r staging"):
                            nc.vector.tensor_scalar_mul(
                                out=dst, in0=po[:, j, 0:HD],
                                scalar1=rcp[:, j:j + 1],
                            )

            spill = []   # closures: previous slab's trailing O-matmuls + drains

            def attend(h, qh, fillers, late_fillers=(), last=False):
                nonlocal spill
                hp, off = h // 2, (h % 2) * HD
                qTp, kTp = qk_tiles[hp]
                pts = pt_pool.tile([128, NKT, 1024], bf16, name="pts", tag="pt")
                po_t = [None, None]
                myspill = []

                def own_o(kt):
                    for qt in range(8):
                        po = po_t[qt // 4]
                        # start=True zeroes the whole 2KB psum bank, so only
                        # the first column of each po bank may assert it
                        nc.tensor.matmul(
                            po[:, qt % 4, :],
                            pts[:, kt, bass.ts(qt, 128)],
                            v_sb[:, kt, h, :],
                            start=(kt == 0 and qt % 4 == 0),
                            stop=(kt == NKT - 1),
                            skip_group_check=True,
                        )

                # early fillers (V / QKV groups — read no drain-produced
                # tiles) spread over j=0..LAG-1; late fillers (transposes,
                # proj — emission-ordered after the j=9,10 drains) over the
                # remaining iterations
                fi = [0, 0]
                flists = (fillers, late_fillers)
                spans = ((0, LAG), (LAG, NKT))

                def emit_fillers(j, which):
                    lo, hi = spans[which]
                    if j < lo:
                        return
                    fl = flists[which]
                    upto = min(
                        (len(fl) * (j - lo + 1) + (hi - lo) - 1) // (hi - lo),
                        len(fl),
                    )
                    while fi[which] < upto:
                        fl[fi[which]]()
                        fi[which] += 1

                for j in range(NKT):
                    pss = pss_pool.tile([128, 1024], f32, name="pss", tag="pss")
                    for c2 in range(2):
                        nc.tensor.matmul(
                            pss[:, bass.ts(c2, 512)],
                            kTp[off:off + HD, bass.ts(j, 128)],
                            qTp[off:off + HD, bass.ds(qh * 1024 + c2 * 512, 512)],
                            start=True, stop=True,
                        )
                    nc.scalar.activation(out=pts[:, j, :], in_=pss, func=ACT_EXP)
                    if j < len(spill):
                        spill[j]()
                    if j == LAG:
                        po_t[0] = po_pool.tile([128, 4, HD + 1], f32, name="po0", tag="po")
                        po_t[1] = po_pool.tile([128, 4, HD + 1], f32, name="po1", tag="po")
                    if j >= LAG:
                        own_o(j - LAG)
                        if last:
                            # no successor S-stream to protect: double-rate
                            # the trailing O-matmuls to shrink the tail
                            own_o(j - LAG + 5)
                    emit_fillers(j, 0)
                    emit_fillers(j, 1)

                if last:
                    # in-loop covered kt0..9 (double rate); tail gets 10..15
                    for kt in range(10, NKT):
                        myspill.append(lambda kt=kt: own_o(kt))
                else:
                    # trailing O-matmuls kt5..15 spill into the next slab at
                    # j=0..8 (two-per-j for the first two), drains at j=9,10
                    # — safely before the po slots are reallocated at j=11
                    myspill.append(lambda: (own_o(NKT - LAG), own_o(NKT - LAG + 1)))
                    myspill.append(lambda: (own_o(NKT - LAG + 2), own_o(NKT - LAG + 3)))
                    for kt in range(NKT - LAG + 4, NKT):
                        myspill.append(lambda kt=kt: own_o(kt))
                myspill.append(lambda: drain_po(h, qh, po_t[0], 0, tail=last))
                myspill.append(lambda: drain_po(h, qh, po_t[1], 4, tail=last))
                spill = myspill

            # ---- static filler schedule -------------------------------
            mk_pair(0)
            FE = {s: [] for s in range(1, 13)}   # early: V / QKV groups
            FL = {s: [] for s in range(1, 13)}   # late: transposes / proj
            FE[1] = (
                [lambda: qkv_group(0, 1, "k")]
                + [lambda kt=kt: v_emit(kt) for kt in range(2)]
                + [lambda: qkv_group(0, 2, "k")]
                + [lambda kt=kt: v_emit(kt) for kt in range(2, 4)]
                + [lambda: qkv_group(0, 3, "k")]
                + [lambda kt=kt: v_emit(kt) for kt in range(4, 9)]
            )
            FE[2] = (
                [lambda kt=kt: v_emit(kt) for kt in range(9, 16)]
                + [lambda: qkv_group(0, 2, "q"),
                   lambda: qkv_group(0, 3, "q")]
            )
            FE[3] = [
                lambda: mk_pair(1),
                lambda: qkv_group(1, 0, "k"),
                lambda: qkv_group(1, 0, "q"),
                lambda: qkv_group(1, 1, "k"),
                lambda: qkv_group(1, 1, "q"),
            ]
            FL[3] = [lambda qt=qt: transpose_qt(0, qt) for qt in range(8)]
            FE[4] = [
                lambda: qkv_group(1, 2, "k"),
                lambda: qkv_group(1, 2, "q"),
                lambda: qkv_group(1, 3, "k"),
                lambda: qkv_group(1, 3, "q"),
            ]
            FE[5] = [
                lambda: mk_pair(2),
                lambda: qkv_group(2, 0, "k"),
                lambda: qkv_group(2, 0, "q"),
            ]
            FL[5] = [lambda qt=qt: transpose_qt(0, qt) for qt in range(8, 16)]
            FE[6] = [
                lambda: qkv_group(2, 1, "k"),
                lambda: qkv_group(2, 1, "q"),
            ]
            FL[7] = [lambda qt=qt: transpose_qt(1, qt) for qt in range(8)]
            FE[7] = [
                lambda: qkv_group(2, 2, "k"),
                lambda: qkv_group(2, 2, "q"),
            ]
            FE[8] = [
                lambda: qkv_group(2, 3, "k"),
                lambda: qkv_group(2, 3, "q"),
            ]
            FL[9] = [lambda qt=qt: transpose_qt(1, qt) for qt in range(8, 16)]
            FL[11] = (
                [lambda qt=qt: transpose_qt(2, qt) for qt in range(8)]
                + [lambda qt=qt, nh=nh: proj_group(qt, nh)
                   for qt in range(4) for nh in range(2)]
            )
            FL[12] = [lambda qt=qt, nh=nh: proj_group(qt, nh)
                      for qt in range(4, 8) for nh in range(2)]

            # ---- startup: pair-0 chunks needed by the first slab, with
            # the three accumulations interleaved by dt so each matmul runs
            # as its xt tile lands (q-ch1 borrows a po slot; aux has 2)
            qTp0, kTp0 = qk_tiles[0]
            ps_k0 = aux_pool.tile([128, 512], f32, name="ps_k0", tag="aux")
            ps_q01 = pss_pool.tile([128, 1024], f32, name="ps_q01", tag="pss")
            for dt in range(NDT):
                nc.tensor.matmul(ps_k0, wk_sb[:, dt, 0:128], xt_dts[dt][:, 0:512],
                                 start=(dt == 0), stop=(dt == NDT - 1))
                nc.tensor.matmul(ps_q01[:, 0:512], wq_sb[:, dt, 0:128],
                                 xt_dts[dt][:, 0:512],
                                 start=(dt == 0), stop=(dt == NDT - 1))
                nc.tensor.matmul(ps_q01[:, 512:1024], wq_sb[:, dt, 0:128],
                                 xt_dts[dt][:, 512:1024],
                                 start=(dt == 0), stop=(dt == NDT - 1))
                if dt < NDT - 1:
                    warm(3)
            # drain the first kt tile's kT columns first so S(kt0) can
            # issue as early as possible, then the q chunks, then the rest
            nc.vector.tensor_scalar_add(out=kTp0[:, 0:128], in0=ps_k0[:, 0:128],
                                        scalar1=bk_sb[:, 0:1])
            nc.vector.tensor_scalar_add(out=qTp0[:, 0:1024], in0=ps_q01,
                                        scalar1=bqs_sb[:, 0:1])
            nc.vector.tensor_scalar_add(out=kTp0[:, 128:512], in0=ps_k0[:, 128:512],
                                        scalar1=bk_sb[:, 0:1])

            for s, (h, qh) in enumerate(SLABS, start=1):
                attend(h, qh, FE[s], FL[s], last=(s == 12))

            # ---- tail -------------------------------------------------
            for fn in spill:
                fn()
            # tail: PE transposes (f32r) run two qt ahead of the projection
            # so the PE→DVE→PE chain pipelines (no HWDGE gen on the
            # critical path); full-width proj psum rides the freed pss
            # slots and the two yr halves drain on DVE and ACT in parallel
            def tail_transpose(qt):
                pst = aux_pool.tile([128, 128], f32r, name="pst", tag="aux")
                nc.tensor.matmul(pst, o2q1_sb[:, qt - 8, :], id_sb,
                                 is_transpose=True)
                dstT = oT_sb[:, 2, bass.ts(qt, 128)]
                if qt % 2 == 1:
                    nc.scalar.activation(out=dstT, in_=pst, func=ACT_COPY)
                else:
                    nc.vector.tensor_copy(out=dstT, in_=pst)

            tail_transpose(8)
            tail_transpose(9)
            for qt in range(8, 16):
                if qt + 2 < 16:
                    tail_transpose(qt + 2)
                # [128, 2, 512]: each nh half owns a full psum bank (matmul
                # accumulation groups must not cross bank boundaries)
                ps = pss_pool.tile([128, 2, 512], f32, name="psyt", tag="pss")
                for nh in range(2):
                    for dtp in range(3):
                        nc.tensor.matmul(
                            ps[:, nh, 0:384], oT_sb[:, dtp, bass.ts(qt, 128)],
                            wp_sb[:, dtp, bass.ts(nh, 384)],
                            start=(dtp == 0), stop=(dtp == 2),
                        )
                yr = yr_pool.tile([128, EMBED], bf16, name="yr", tag="yr")
                nc.vector.tensor_copy(out=yr[:, 0:384], in_=ps[:, 0, 0:384])
                nc.scalar.activation(out=yr[:, 384:768], in_=ps[:, 1, 0:384],
                                     func=ACT_COPY)
                nc.sync.dma_start(out=y_d.ap()[bass.ts(qt, 128), :], in_=yr)

    nc.finalize()
    return nc


def _shard_inputs(x, w_qkv, b_qkv, w_proj):
    import ml_dtypes

    bf16 = ml_dtypes.bfloat16
    in_maps = []
    for c in range(NCORES):
        b, g = c // 2, c % 2
        sl = slice(DL * g, DL * g + DL)
        in_maps.append({
            "xt": np.ascontiguousarray(x[b].T).astype(bf16),
            "wq": np.ascontiguousarray(w_qkv[:, sl] * SCALE).astype(bf16),
            "wk": np.ascontiguousarray(w_qkv[:, EMBED:][:, sl]).astype(bf16),
            "wv": np.ascontiguousarray(w_qkv[:, 2 * EMBED:][:, sl]).astype(bf16),
            "bqs": np.ascontiguousarray(b_qkv[sl] * SCALE),
            "bk": np.ascontiguousarray(b_qkv[EMBED:][sl]),
            "wp": np.ascontiguousarray(w_proj[sl, :]).astype(bf16),
            "ident": np.eye(128, dtype=np.float32),
        })
    return in_maps


def kernel(x, w_qkv, b_qkv, w_proj, b_proj, _profile=False, _repeat=1):
    from concourse.bass_utils import run_bass_kernel_spmd

    x = np.asarray(x, dtype=np.float32)
    w_qkv = np.asarray(w_qkv, dtype=np.float32)
    b_qkv = np.asarray(b_qkv, dtype=np.float32)
    w_proj = np.asarray(w_proj, dtype=np.float32)
    b_proj = np.asarray(b_proj, dtype=np.float32)

    if _repeat not in _prog_cache:
        _prog_cache[_repeat] = _build_program(_repeat)
    nc = _prog_cache[_repeat]

    in_maps = _shard_inputs(x, w_qkv, b_qkv, w_proj)
    res = run_bass_kernel_spmd(
        nc, in_maps, list(range(NCORES)), trace=_profile,
    )

    # host-side gather: sum the two head-group partials per batch and add
    # the bias row (v-bias folded through w_proj, plus b_proj itself)
    bias_row = b_qkv[2 * EMBED:] @ w_proj + b_proj
    y = np.empty((B, T, EMBED), dtype=np.float32)
    for b in range(B):
        y[b] = (res.results[2 * b]["y"].astype(np.float32)
                + res.results[2 * b + 1]["y"].astype(np.float32) + bias_row)
    if _profile:
        return y, res
    return y


# revision 58
# speedup vs baseline: 1.0091x; 1.0047x over previous
"""Multi-head attention (B=4, T=2048, D=768, H=12) on 8 NeuronCores.

Sharding: core c handles batch b = c//2 and head-group g = c%2 (heads
6g..6g+5).  Each core computes its 6 heads' attention and a partial
output projection (contraction over its 384 local dims of w_proj); the
host sums the two partials per batch and adds the bias row.

Device formulation (bf16 matmul operands everywhere, fp32 psum):
  qT = Wq'.T @ xT  [384, 2048]   (Wq' pre-scaled by 1/sqrt(hd) on host)
  kT = Wk.T @ xT   [384, 2048]
  v  = x @ Wv      [2048, 384]   per kpos tile (65th column = 1.0)
  S^T[kt] = kT_h.T @ qT_h   [128 kpos, 1024 q]  per head, q-slab halves
  P^T = exp(S^T)   (ACT; scores max ~8 so no max subtraction)
  O[q, 65] += P^T[kt].T @ v'[kt]   <- flipped: output partitions = 128 q
      (col 64 accumulates the softmax denominators via the ones column)
  o = O[:, 0:64] * rcp(O[:, 64])   fused normalize in the psum drain
  oT via PE transpose (identity stationary), y = oT.T @ Wp per q tile.

The flip halves the P@V matmul cost vs the [65, 512]-output orientation
(the timeline cost model charges out-free-size cycles per matmul, so
output partition utilization is what matters).  Schedule: ACT (exp)
paces the attention inner loop at ~1.04us per [128,1024] tile; S, O,
QKV-projection, V, transposes and the output projection are spread
across the 12 (head, q-slab) sweeps to keep PE under that pace.
O-matmuls trail their exp by 4 kt iterations so psum-slot drains (DVE)
never stall the PE queue head.
"""

import numpy as np

EMBED = 768
HEADS = 12
HD = 64
SCALE = HD ** -0.5
B, T = 4, 2048
NCORES = 8
HPC = 6            # heads per core
DL = HPC * HD      # 384 local model dims per core

_prog_cache = {}


def _build_program(repeat=1):
    import concourse.bass as bass
    import concourse.mybir as mybir
    import concourse.tile as tile
    from concourse import bacc

    f32 = mybir.dt.float32
    f32r = mybir.dt.float32r
    bf16 = mybir.dt.bfloat16
    ACT_EXP = mybir.ActivationFunctionType.Exp
    ACT_COPY = mybir.ActivationFunctionType.Copy

    nc = bacc.Bacc()

    xt_d = nc.dram_tensor("xt", [EMBED, T], bf16, kind="ExternalInput")
    wq_d = nc.dram_tensor("wq", [EMBED, DL], bf16, kind="ExternalInput")
    wk_d = nc.dram_tensor("wk", [EMBED, DL], bf16, kind="ExternalInput")
    wv_d = nc.dram_tensor("wv", [EMBED, DL], bf16, kind="ExternalInput")
    bqs_d = nc.dram_tensor("bqs", [DL], f32, kind="ExternalInput")
    bk_d = nc.dram_tensor("bk", [DL], f32, kind="ExternalInput")
    wp_d = nc.dram_tensor("wp", [DL, EMBED], bf16, kind="ExternalInput")
    id_d = nc.dram_tensor("ident", [128, 128], f32r, kind="ExternalInput")
    y_d = nc.dram_tensor("y", [T, EMBED], bf16, kind="ExternalOutput")

    NDT = EMBED // 128   # 6 contraction tiles over embed dim
    NKT = T // 128       # 16 key-position tiles
    NQT = T // 128       # 16 query row tiles
    LAG = 11             # O-matmul lag (in kt iterations) behind exp: each
    #                      slab's kt5..15 O-matmuls spill into the next slab,
    #                      spreading V/QKV pressure out of the first slabs

    # (head, q-slab) sweep order: q-major within each head pair so a
    # pair's q-half completes as early as possible (feeds transposes).
    SLABS = [(0, 0), (1, 0), (0, 1), (1, 1),
             (2, 0), (3, 0), (2, 1), (3, 1),
             (4, 0), (5, 0), (4, 1), (5, 1)]

    with tile.TileContext(nc) as tc:
      for _rep in range(repeat):
        with tc.tile_pool(name="pers", bufs=1) as pers, \
             tc.tile_pool(name="qk", bufs=2) as qk_pool, \
             tc.tile_pool(name="pt", bufs=2) as pt_pool, \
             tc.tile_pool(name="rcp", bufs=4) as rcp_pool, \
             tc.tile_pool(name="yr", bufs=3) as yr_pool, \
             tc.tile_pool(name="pss", bufs=2, space="PSUM") as pss_pool, \
             tc.tile_pool(name="po", bufs=2, space="PSUM") as po_pool, \
             tc.tile_pool(name="aux", bufs=2, space="PSUM") as aux_pool:

            xt_dts = [pers.tile([128, T], bf16, name=f"xt{dt}_sb")
                      for dt in range(NDT)]
            wq_sb = pers.tile([128, NDT, DL], bf16, name="wq_sb")
            wk_sb = pers.tile([128, NDT, DL], bf16, name="wk_sb")
            wv_sb = pers.tile([128, NDT, DL], bf16, name="wv_sb")
            wp_sb = pers.tile([128, 3, EMBED], bf16, name="wp_sb")
            v_sb = pers.tile([128, NKT, HPC, HD + 1], bf16, name="v_sb")
            bqs_sb = pers.tile([128, 3], f32, name="bqs_sb")
            bk_sb = pers.tile([128, 3], f32, name="bk_sb")
            o_sb = pers.tile([128, NQT, 3, 128], bf16, name="o_sb")
            # pair-2 qh1 stages in f32r so the tail can transpose on the PE
            # (no serial HWDGE descriptor-gens on the critical tail path)
            o2q1_sb = pers.tile([128, 8, 128], f32r, name="o2q1_sb")
            id_sb = pers.tile([128, 128], f32r, name="id_sb")
            oT_sb = pers.tile([128, 3, T], bf16, name="oT_sb")
            warm_sb = pers.tile([128, 256], bf16, name="warm_sb")

            # ones column of v' (softmax denominator accumulator) — only
            # the 65th columns; emitted first so it doesn't sit behind the
            # DMA descriptor generation on the Pool queue
            nc.gpsimd.memset(v_sb[:, :, :, HD:HD + 1], 1.0)
            nc.vector.memset(warm_sb, 0.0)

            # input DMAs: first-slab critical path is all of xt (full embed
            # contraction) + the pair-0 columns of wk/wq.  The DMA copies
            # serialize on one resource, so everything else goes after xt.
            # Per-dt xt tiles give each transfer its own completion sem
            # (DMA write deps are tile x queue granular).
            nc.gpsimd.dma_start(
                out=wk_sb[:, :, 0:128],
                in_=wk_d.ap()[:, 0:128].rearrange("(n p) m -> p n m", p=128))
            nc.gpsimd.dma_start(
                out=wq_sb[:, :, 0:128],
                in_=wq_d.ap()[:, 0:128].rearrange("(n p) m -> p n m", p=128))
            for dt in range(3):
                nc.sync.dma_start(out=xt_dts[dt], in_=xt_d.ap()[bass.ts(dt, 128), :])
            for dt in range(3, NDT):
                nc.gpsimd.dma_start(out=xt_dts[dt], in_=xt_d.ap()[bass.ts(dt, 128), :])
            nc.sync.dma_start(out=bqs_sb, in_=bqs_d.ap().rearrange("(n p) -> p n", p=128))
            nc.sync.dma_start(out=bk_sb, in_=bk_d.ap().rearrange("(n p) -> p n", p=128))
            nc.gpsimd.dma_start(out=wv_sb, in_=wv_d.ap().rearrange("(n p) m -> p n m", p=128))
            nc.gpsimd.dma_start(
                out=wk_sb[:, :, 128:DL],
                in_=wk_d.ap()[:, 128:DL].rearrange("(n p) m -> p n m", p=128))
            nc.gpsimd.dma_start(
                out=wq_sb[:, :, 128:DL],
                in_=wq_d.ap()[:, 128:DL].rearrange("(n p) m -> p n m", p=128))
            nc.gpsimd.dma_start(out=wp_sb, in_=wp_d.ap().rearrange("(n p) m -> p n m", p=128))
            nc.gpsimd.dma_start(out=id_sb, in_=id_d.ap())

            def warm(n):
                # warm-up matmuls ride the po slots (idle until the first
                # slab's O accumulation; pss holds ps_q01 through startup)
                for _w in range(n):
                    psw = po_pool.tile([128, 256], f32, name="psw", tag="po")
                    nc.tensor.matmul(psw, warm_sb[0:2, 0:128], warm_sb[0:2, :],
                                     start=True, stop=True)

            warm(6)

            qk_tiles = {}
            yr_tiles = {}

            def mk_pair(hp):
                qk_tiles[hp] = (
                    qk_pool.tile([128, T], bf16, name="qTp", tag="qT"),
                    qk_pool.tile([128, T], bf16, name="kTp", tag="kT"),
                )

            def qkv_group(hp, ch, which):
                csl = bass.ts(ch, 512)
                qTp, kTp = qk_tiles[hp]
                dst, wsb, bias = (
                    (qTp, wq_sb, bqs_sb) if which == "q" else (kTp, wk_sb, bk_sb)
                )
                ps = aux_pool.tile([128, 512], f32, name="psqk", tag="aux")
                for dt in range(NDT):
                    nc.tensor.matmul(
                        ps, wsb[:, dt, bass.ts(hp, 128)], xt_dts[dt][:, csl],
                        start=(dt == 0), stop=(dt == NDT - 1),
                    )
                nc.vector.tensor_scalar_add(
                    out=dst[:, csl], in0=ps, scalar1=bias[:, hp:hp + 1],
                )

            def v_emit(kt):
                ps = aux_pool.tile([128, DL], f32, name="psv", tag="aux")
                for dt in range(NDT):
                    nc.tensor.matmul(
                        ps, xt_dts[dt][:, bass.ts(kt, 128)], wv_sb[:, dt, :],
                        start=(dt == 0), stop=(dt == NDT - 1),
                    )
                # GPSIMD cannot touch PSUM; DVE is nearly idle during the
                # V-emission slab (ACT copies here would stall its in-order
                # queue ahead of the exps)
                nc.vector.tensor_copy(
                    out=v_sb[:, kt, :, 0:HD],
                    in_=ps.rearrange("p (h d) -> p h d", h=HPC),
                )

            def transpose_qt(pair, qt, tail=False):
                # 2-byte dtypes transpose on the DMA xbar (PE transpose into
                # psum is 4-byte-cell granular and corrupts bf16).  Tail
                # transposes issue from the otherwise-idle ACT queue so their
                # descriptor generation doesn't serialize behind the y DMAs.
                eng = nc.scalar if tail else nc.sync
                eng.dma_start_transpose(
                    out=oT_sb[:, pair, bass.ts(qt, 128)],
                    in_=o_sb[:, qt, pair, :],
                )

            def proj_group(qt, nh, tail=False):
                ps = aux_pool.tile([128, 384], f32, name="psy", tag="aux")
                for dtp in range(3):
                    nc.tensor.matmul(
                        ps, oT_sb[:, dtp, bass.ts(qt, 128)],
                        wp_sb[:, dtp, bass.ts(nh, 384)],
                        start=(dtp == 0), stop=(dtp == 2),
                    )
                if nh == 0:
                    yr = yr_pool.tile([128, EMBED], bf16, name="yr", tag="yr")
                    yr_tiles[qt] = yr
                    nc.vector.tensor_copy(out=yr[:, 0:384], in_=ps)
                else:
                    yr = yr_tiles.pop(qt)
                    if tail:  # ACT is idle once attention has drained
                        nc.scalar.activation(out=yr[:, 384:768], in_=ps, func=ACT_COPY)
                    else:
                        nc.vector.tensor_copy(out=yr[:, 384:768], in_=ps)
                    nc.sync.dma_start(out=y_d.ap()[bass.ts(qt, 128), :], in_=yr)

            def drain_po(h, qh, po, qt_base, tail=False):
                # fused normalize: o = O[:, 0:64] / O[:, 64] at psum drain.
                # In the tail (ACT idle) half the copies go through ACT's
                # Copy-with-scale to unserialize the DVE.
                pair, off = h // 2, (h % 2) * HD
                rcp = rcp_pool.tile([128, 4], f32, name="rcp", tag="rcp")
                nc.vector.reciprocal(out=rcp, in_=po[:, :, HD])
                for j in range(4):
                    qt = qh * 8 + qt_base + j
                    if pair == 2 and qh == 1:
                        dst = o2q1_sb[:, qt - 8, off:off + HD]
                    else:
                        dst = o_sb[:, qt, pair, off:off + HD]
                    if tail and j % 2 == 1:
                        nc.scalar.activation(
                            out=dst, in_=po[:, j, 0:HD], func=ACT_COPY,
                            scale=rcp[:, j:j + 1],
                        )
                    else:
                        with nc.allow_low_precision(reason="f32r staging"):
                            nc.vector.tensor_scalar_mul(
                                out=dst, in0=po[:, j, 0:HD],
                                scalar1=rcp[:, j:j + 1],
                            )

            spill = []   # closures: previous slab's trailing O-matmuls + drains

            def attend(h, qh, fillers, late_fillers=(), last=False):
                nonlocal spill
                hp, off = h // 2, (h % 2) * HD
                qTp, kTp = qk_tiles[hp]
                pts = pt_pool.tile([128, NKT, 1024], bf16, name="pts", tag="pt")
                po_t = [None, None]
                myspill = []

                def own_o(kt):
                    for qt in range(8):
                        po = po_t[qt // 4]
                        # start=True zeroes the whole 2KB psum bank, so only
                        # the first column of each po bank may assert it
                        nc.tensor.matmul(
                            po[:, qt % 4, :],
                            pts[:, kt, bass.ts(qt, 128)],
                            v_sb[:, kt, h, :],
                            start=(kt == 0 and qt % 4 == 0),
                            stop=(kt == NKT - 1),
                            skip_group_check=True,
                        )

                # early fillers (V / QKV groups — read no drain-produced
                # tiles) spread over j=0..LAG-1; late fillers (transposes,
                # proj — emission-ordered after the j=9,10 drains) over the
                # remaining iterations
                fi = [0, 0]
                flists = (fillers, late_fillers)
                spans = ((0, LAG), (LAG, NKT))

                def emit_fillers(j, which):
                    lo, hi = spans[which]
                    if j < lo:
                        return
                    fl = flists[which]
                    upto = min(
                        (len(fl) * (j - lo + 1) + (hi - lo) - 1) // (hi - lo),
                        len(fl),
                    )
                    while fi[which] < upto:
                        fl[fi[which]]()
                        fi[which] += 1

                for j in range(NKT):
                    pss = pss_pool.tile([128, 1024], f32, name="pss", tag="pss")
                    for c2 in range(2):
                        nc.tensor.matmul(
                            pss[:, bass.ts(c2, 512)],
                            kTp[off:off + HD, bass.ts(j, 128)],
                            qTp[off:off + HD, bass.ds(qh * 1024 + c2 * 512, 512)],
                            start=True, stop=True,
                        )
                    nc.scalar.activation(out=pts[:, j, :], in_=pss, func=ACT_EXP)
                    if j < len(spill):
                        spill[j]()
                    if j == LAG:
                        po_t[0] = po_pool.tile([128, 4, HD + 1], f32, name="po0", tag="po")
                        po_t[1] = po_pool.tile([128, 4, HD + 1], f32, name="po1", tag="po")
                    if j >= LAG:
                        own_o(j - LAG)
                        if last:
                            # no successor S-stream to protect: double-rate
                            # the trailing O-matmuls to shrink the tail
                            own_o(j - LAG + 5)
                    emit_fillers(j, 0)
                    emit_fillers(j, 1)

                if last:
                    # in-loop covered kt0..9 (double rate); tail gets 10..15
                    for kt in range(10, NKT):
                        myspill.append(lambda kt=kt: own_o(kt))
                else:
                    # trailing O-matmuls kt5..15 spill into the next slab at
                    # j=0..8 (two-per-j for the first two), drains at j=9,10
                    # — safely before the po slots are reallocated at j=11
                    myspill.append(lambda: (own_o(NKT - LAG), own_o(NKT - LAG + 1)))
                    myspill.append(lambda: (own_o(NKT - LAG + 2), own_o(NKT - LAG + 3)))
                    for kt in range(NKT - LAG + 4, NKT):
                        myspill.append(lambda kt=kt: own_o(kt))
                myspill.append(lambda: drain_po(h, qh, po_t[0], 0, tail=last))
                myspill.append(lambda: drain_po(h, qh, po_t[1], 4, tail=last))
                spill = myspill

            # ---- static filler schedule -------------------------------
            mk_pair(0)
            FE = {s: [] for s in range(1, 13)}   # early: V / QKV groups
            FL = {s: [] for s in range(1, 13)}   # late: transposes / proj
            FE[1] = (
                [lambda: qkv_group(0, 1, "k")]
                + [lambda kt=kt: v_emit(kt) for kt in range(2)]
                + [lambda: qkv_group(0, 2, "k")]
                + [lambda kt=kt: v_emit(kt) for kt in range(2, 4)]
                + [lambda: qkv_group(0, 3, "k")]
                + [lambda kt=kt: v_emit(kt) for kt in range(4, 9)]
            )
            FE[2] = (
                [lambda kt=kt: v_emit(kt) for kt in range(9, 16)]
                + [lambda: qkv_group(0, 2, "q"),
                   lambda: qkv_group(0, 3, "q")]
            )
            FE[3] = [
                lambda: mk_pair(1),
                lambda: qkv_group(1, 0, "k"),
                lambda: qkv_group(1, 0, "q"),
                lambda: qkv_group(1, 1, "k"),
                lambda: qkv_group(1, 1, "q"),
            ]
            FL[3] = [lambda qt=qt: transpose_qt(0, qt) for qt in range(8)]
            FE[4] = [
                lambda: qkv_group(1, 2, "k"),
                lambda: qkv_group(1, 2, "q"),
                lambda: qkv_group(1, 3, "k"),
                lambda: qkv_group(1, 3, "q"),
            ]
            FE[5] = [
                lambda: mk_pair(2),
                lambda: qkv_group(2, 0, "k"),
                lambda: qkv_group(2, 0, "q"),
            ]
            FL[5] = [lambda qt=qt: transpose_qt(0, qt) for qt in range(8, 16)]
            FE[6] = [
                lambda: qkv_group(2, 1, "k"),
                lambda: qkv_group(2, 1, "q"),
            ]
            FL[7] = [lambda qt=qt: transpose_qt(1, qt) for qt in range(8)]
            FE[7] = [
                lambda: qkv_group(2, 2, "k"),
                lambda: qkv_group(2, 2, "q"),
            ]
            FE[8] = [
                lambda: qkv_group(2, 3, "k"),
                lambda: qkv_group(2, 3, "q"),
            ]
            FL[9] = [lambda qt=qt: transpose_qt(1, qt) for qt in range(8, 16)]
            FL[11] = (
                [lambda qt=qt: transpose_qt(2, qt) for qt in range(8)]
                + [lambda qt=qt, nh=nh: proj_group(qt, nh)
                   for qt in range(4) for nh in range(2)]
            )
            FL[12] = [lambda qt=qt, nh=nh: proj_group(qt, nh)
                      for qt in range(4, 8) for nh in range(2)]

            # ---- startup: pair-0 chunks needed by the first slab, with
            # the three accumulations interleaved by dt so each matmul runs
            # as its xt tile lands (q-ch1 borrows a po slot; aux has 2)
            qTp0, kTp0 = qk_tiles[0]
            ps_k0 = aux_pool.tile([128, 512], f32, name="ps_k0", tag="aux")
            ps_q01 = pss_pool.tile([128, 1024], f32, name="ps_q01", tag="pss")
            for dt in range(NDT):
                nc.tensor.matmul(ps_k0, wk_sb[:, dt, 0:128], xt_dts[dt][:, 0:512],
                                 start=(dt == 0), stop=(dt == NDT - 1))
                nc.tensor.matmul(ps_q01[:, 0:512], wq_sb[:, dt, 0:128],
                                 xt_dts[dt][:, 0:512],
                                 start=(dt == 0), stop=(dt == NDT - 1))
                nc.tensor.matmul(ps_q01[:, 512:1024], wq_sb[:, dt, 0:128],
                                 xt_dts[dt][:, 512:1024],
                                 start=(dt == 0), stop=(dt == NDT - 1))
                if dt < NDT - 1:
                    warm(3)
            # drain the first kt tile's kT columns first so S(kt0) can
            # issue as early as possible, then the q halves, then the rest
            nc.vector.tensor_scalar_add(out=kTp0[:, 0:128], in0=ps_k0[:, 0:128],
                                        scalar1=bk_sb[:, 0:1])
            nc.vector.tensor_scalar_add(out=qTp0[:, 0:512], in0=ps_q01[:, 0:512],
                                        scalar1=bqs_sb[:, 0:1])
            nc.vector.tensor_scalar_add(out=qTp0[:, 512:1024], in0=ps_q01[:, 512:1024],
                                        scalar1=bqs_sb[:, 0:1])
            nc.vector.tensor_scalar_add(out=kTp0[:, 128:512], in0=ps_k0[:, 128:512],
                                        scalar1=bk_sb[:, 0:1])

            for s, (h, qh) in enumerate(SLABS, start=1):
                attend(h, qh, FE[s], FL[s], last=(s == 12))

            # ---- tail -------------------------------------------------
            for fn in spill:
                fn()
            # tail: PE transposes (f32r) run two qt ahead of the projection
            # so the PE→DVE→PE chain pipelines (no HWDGE gen on the
            # critical path); full-width proj psum rides the freed pss
            # slots and the two yr halves drain on DVE and ACT in parallel
            def tail_transpose(qt):
                pst = aux_pool.tile([128, 128], f32r, name="pst", tag="aux")
                nc.tensor.matmul(pst, o2q1_sb[:, qt - 8, :], id_sb,
                                 is_transpose=True)
                dstT = oT_sb[:, 2, bass.ts(qt, 128)]
                if qt % 2 == 1:
                    nc.scalar.activation(out=dstT, in_=pst, func=ACT_COPY)
                else:
                    nc.vector.tensor_copy(out=dstT, in_=pst)

            tail_transpose(8)
            tail_transpose(9)
            for qt in range(8, 16):
                if qt + 2 < 16:
                    tail_transpose(qt + 2)
                # [128, 2, 512]: each nh half owns a full psum bank (matmul
                # accumulation groups must not cross bank boundaries)
                ps = pss_pool.tile([128, 2, 512], f32, name="psyt", tag="pss")
                for nh in range(2):
                    for dtp in range(3):
                        nc.tensor.matmul(
                            ps[:, nh, 0:384], oT_sb[:, dtp, bass.ts(qt, 128)],
                            wp_sb[:, dtp, bass.ts(nh, 384)],
                            start=(dtp == 0), stop=(dtp == 2),
                        )
                yr = yr_pool.tile([128, EMBED], bf16, name="yr", tag="yr")
                nc.vector.tensor_copy(out=yr[:, 0:384], in_=ps[:, 0, 0:384])
                nc.scalar.activation(out=yr[:, 384:768], in_=ps[:, 1, 0:384],
                                     func=ACT_COPY)
                nc.sync.dma_start(out=y_d.ap()[bass.ts(qt, 128), :], in_=yr)

    nc.finalize()
    return nc


def _shard_inputs(x, w_qkv, b_qkv, w_proj):
    import ml_dtypes

    bf16 = ml_dtypes.bfloat16
    in_maps = []
    for c in range(NCORES):
        b, g = c // 2, c % 2
        sl = slice(DL * g, DL * g + DL)
        in_maps.append({
            "xt": np.ascontiguousarray(x[b].T).astype(bf16),
            "wq": np.ascontiguousarray(w_qkv[:, sl] * SCALE).astype(bf16),
            "wk": np.ascontiguousarray(w_qkv[:, EMBED:][:, sl]).astype(bf16),
            "wv": np.ascontiguousarray(w_qkv[:, 2 * EMBED:][:, sl]).astype(bf16),
            "bqs": np.ascontiguousarray(b_qkv[sl] * SCALE),
            "bk": np.ascontiguousarray(b_qkv[EMBED:][sl]),
            "wp": np.ascontiguousarray(w_proj[sl, :]).astype(bf16),
            "ident": np.eye(128, dtype=np.float32),
        })
    return in_maps


def kernel(x, w_qkv, b_qkv, w_proj, b_proj, _profile=False, _repeat=1):
    from concourse.bass_utils import run_bass_kernel_spmd

    x = np.asarray(x, dtype=np.float32)
    w_qkv = np.asarray(w_qkv, dtype=np.float32)
    b_qkv = np.asarray(b_qkv, dtype=np.float32)
    w_proj = np.asarray(w_proj, dtype=np.float32)
    b_proj = np.asarray(b_proj, dtype=np.float32)

    if _repeat not in _prog_cache:
        _prog_cache[_repeat] = _build_program(_repeat)
    nc = _prog_cache[_repeat]

    in_maps = _shard_inputs(x, w_qkv, b_qkv, w_proj)
    res = run_bass_kernel_spmd(
        nc, in_maps, list(range(NCORES)), trace=_profile,
    )

    # host-side gather: sum the two head-group partials per batch and add
    # the bias row (v-bias folded through w_proj, plus b_proj itself)
    bias_row = b_qkv[2 * EMBED:] @ w_proj + b_proj
    y = np.empty((B, T, EMBED), dtype=np.float32)
    for b in range(B):
        y[b] = (res.results[2 * b]["y"].astype(np.float32)
                + res.results[2 * b + 1]["y"].astype(np.float32) + bias_row)
    if _profile:
        return y, res
    return y
